# revision 20
# baseline (speedup 1.0000x reference)
"""Trainium2 Bass kernel for nn_BotRGCN2 (2-layer RGCN over 100k nodes / 600k edges).

Strategy (8 NeuronCores, SPMD):
  - Shard nodes across cores (12500/core, padded to 12544 = 98 windows of 128).
  - Feature-major (transposed) activations on-chip; node-major gather table in
    DRAM.
  - Gather-first RGCN: AllGather the raw x shards (node-major, bf16) into a
    full [C*NP2, 128] table per layer; per owned 128-node window, dma_gather
    the per-edge source rows and scatter-add them on the PE:
    psum[feat, slot] += G^T @ S with lhsT = G (gathered rows
    [128 edges x 128 feat]) and rhs = S ([128 edges x 256 slots], slot =
    rel*128 + dst_local, value 1/cnt(dst,rel)).  S is built ON DEVICE by the
    DVE from a 4-byte/edge (slot, weight) stream: S = (iota == slot) * w.
    Per-relation weights are applied AFTER aggregation (mean is linear):
    x_new = sum_r W_r^T @ mean_r + root^T @ x + bias  (3 matmuls/window).
  - Edges preprocessed on host: partitioned by dst owner, grouped by
    (window, src-owner-block), padded to 128-edge chunks with weight-0
    entries; chunk structure shared by all 8 cores (max over cores) so one
    SPMD program serves every core.  Gathers capped at 1024 indices and
    round-robined over 4 SWDGE queues.  Index and slot/weight streams are
    identical for both layers and loaded once.
"""

import sys
from contextlib import ExitStack

import numpy as np

sys.path.insert(0, "/opt/trn_rl_repo")

import ml_dtypes  # noqa: E402
import concourse.bass as bass  # noqa: E402,F401
import concourse.bacc as bacc  # noqa: E402
import concourse.mybir as mybir  # noqa: E402
import concourse.tile as tile  # noqa: E402
from concourse import library_config  # noqa: E402
from concourse.bass_utils import run_bass_kernel_spmd  # noqa: E402
from concourse.instruction_name_ordered_set import (  # noqa: E402
    InstructionNameOrderedSet,
)


def _order(after_bi, befores):
    """Anchor `after_bi` behind `befores` with no-sync (ordering-only) edges
    so the Tile scheduler cannot reorder user-synced semaphore protocol."""
    deps = InstructionNameOrderedSet()
    for b in befores:
        if b is not None:
            deps.add(b.ins.name)
    if deps:
        after_bi.ins.add_nosync_dependencies_from(deps)

C = 8           # cores
WIN = 128       # dst nodes per PSUM window
R = 2           # relations

# tunables
SG_WINDOWS = 8       # windows per gather supergroup
NIDX_CHUNKS_MAX = 8  # max 128-idx chunks per dma_gather (carveout limit)
SINGLE_PACKET = True
PREP_ONLY = True    # prepare_only + trigger_dma path
PIPE_DEPTH = 1      # supergroups the Pool queue may run ahead of the PE
SCALAR_MOD = 5      # 1 in SCALAR_MOD S-builds go to the scalar engine
G_BUFS = 0           # 0 = auto (gather insts per supergroup + headroom)
S_BUFS = 32          # on-device one-hot S tiles in flight
PS_BUFS = 4
PSB_BUFS = 3
M_BUFS = 4
N_QUEUES = 4         # SWDGE queues; gathers round-robin across them
FW0 = 4096           # stage-1 DMA tile width (8KB descriptors)
USE_BF16 = True      # bf16 activations (messages are always bf16)
TRACE = False
TMPDIR = None

F32 = mybir.dt.float32
BF16 = mybir.dt.bfloat16
LAST_RESULTS = None  # BassKernelResults of the most recent run


def _dt():
    return BF16 if USE_BF16 else F32


def _np_dt():
    return ml_dtypes.bfloat16 if USE_BF16 else np.float32


def _col_tiles(total, width):
    out = []
    c = 0
    while c < total:
        out.append((c, min(width, total - c)))
        c += width
    return out


# ----------------------------------------------------------------------------
# host-side edge preprocessing
# ----------------------------------------------------------------------------

def _edge_meta(src, dst, et, N, NPC, NP2):
    """Build the SPMD-uniform chunk structure, per-core gather index streams,
    and per-core (slot, weight) streams for on-device one-hot S tiles."""
    E = src.shape[0]
    NW = NP2 // WIN
    seg = dst * R + et
    cnt = np.bincount(seg, minlength=N * R).astype(np.float64)
    w = (1.0 / np.maximum(cnt, 1.0))[seg]

    core = dst // NPC
    nl = dst % NPC
    vwin = nl // WIN
    dloc = nl % WIN
    blk = src // NPC                       # table block == src owner core
    tloc = src % NPC                       # row within block (< NP2 <= 32767)
    slot = et * WIN + dloc                 # 0..2*WIN-1
    assert NP2 <= 32768

    counts = np.zeros((C, NW, C), np.int64)
    np.add.at(counts, (core, vwin, blk), 1)
    K = -(-counts.max(axis=0) // WIN)      # [NW, C] chunks per (win, blk)

    # compute-order chunk bases (v-major, then b, then k)
    co_base = np.zeros((NW, C), np.int64)
    cc = 0
    for v in range(NW):
        for b in range(C):
            co_base[v, b] = cc
            cc += K[v, b]
    TC = int(cc)

    # gather-order (supergroup, block, window, k) + gather instruction list
    go_base = np.zeros((NW, C), np.int64)
    gather_insts = []  # (blk, start_chunk, n_chunks, sg_start)
    gc = 0
    for s0 in range(0, NW, SG_WINDOWS):
        vs = range(s0, min(s0 + SG_WINDOWS, NW))
        for b in range(C):
            nch = int(sum(int(K[v, b]) for v in vs))
            if nch == 0:
                continue
            off = 0
            while off < nch:
                n = min(NIDX_CHUNKS_MAX, nch - off)
                gather_insts.append((b, gc + off, n, s0))
                off += n
            for v in vs:
                go_base[v, b] = gc
                gc += K[v, b]
    assert gc == TC

    # per-core streams
    order = np.argsort((core * NW + vwin) * C + blk, kind="stable")
    gid = ((core * NW + vwin) * C + blk)[order]
    starts = np.concatenate([[0], np.cumsum(np.bincount(gid, minlength=C * NW * C))])
    rank = np.arange(E) - starts[gid]

    ce = core[order]
    v_ = vwin[order]
    b_ = blk[order]
    k_ = rank // WIN
    lane = rank % WIN

    idxg = np.zeros((C, TC * WIN), np.int16)
    gpos = (go_base[v_, b_] + k_) * WIN + lane
    idxg[ce, gpos] = tloc[order].astype(np.int16)

    # per-edge (slot, w, -slot, -w) in compute order: lane-major [C, WIN, TC, 4]
    slotw = np.zeros((C, WIN, TC, 4), np.float32)
    cchunk = co_base[v_, b_] + k_
    slotw[ce, lane, cchunk, 0] = slot[order]
    slotw[ce, lane, cchunk, 1] = w[order].astype(np.float32)
    slotw[ce, lane, cchunk, 2] = -slot[order]
    slotw[ce, lane, cchunk, 3] = -w[order].astype(np.float32)

    # wrap indices per gather instruction: idx i -> [i%16, off + i//16]
    TIDX = TC * WIN
    idxw = np.zeros((C, 128, TIDX // 16), np.int16)
    for (b, gc0, nch, s0) in gather_insts:
        n = nch * WIN
        segm = idxg[:, gc0 * WIN: gc0 * WIN + n].reshape(C, n // 16, 16)
        idxw[:, :16, gc0 * 8: gc0 * 8 + n // 16] = segm.transpose(0, 2, 1)
    idxw[:, 16:, :] = np.tile(idxw[:, :16, :], (1, 7, 1))

    return dict(K=K, co_base=co_base, go_base=go_base,
                gather_insts=gather_insts,
                TC=TC, TIDX=TIDX, NW=NW, idxw=idxw, slotw=slotw)


# ----------------------------------------------------------------------------
# device program
# ----------------------------------------------------------------------------

def _build_program(shapes, meta):
    DT = _dt()
    N, TW, D, OUT, NPC, NP2 = (shapes[k] for k in
                               ("N", "TW", "D", "OUT", "NPC", "NP2"))
    KT = TW // 128
    NW = meta["NW"]
    TC, TIDX = meta["TC"], meta["TIDX"]
    K, co_base, go_base = meta["K"], meta["co_base"], meta["go_base"]
    gather_insts = meta["gather_insts"]
    AF = mybir.ActivationFunctionType
    ALU = mybir.AluOpType

    nc = bacc.Bacc("TRN2", target_bir_lowering=False,
                   num_swdge_queues=N_QUEUES)

    twT = nc.dram_tensor("twT", [128, KT, NP2], DT, kind="ExternalInput")
    idx16 = nc.dram_tensor("idx16", [128, TIDX // 16], mybir.dt.int16,
                           kind="ExternalInput")
    slotw = nc.dram_tensor("slotw", [128, TC, 4], F32, kind="ExternalInput")
    iota = nc.dram_tensor("iota", [128, R * WIN], BF16, kind="ExternalInput")
    ident = nc.dram_tensor("ident", [128, 128], DT, kind="ExternalInput")
    wt = nc.dram_tensor("wt", [128, KT, 128], DT, kind="ExternalInput")
    bt = nc.dram_tensor("bt", [128, 1], F32, kind="ExternalInput")
    win = nc.dram_tensor("win", [128, 128], DT, kind="ExternalInput")
    bin_ = nc.dram_tensor("bin", [128, 1], F32, kind="ExternalInput")
    wr = nc.dram_tensor("wr", [128, R * 128], DT, kind="ExternalInput")
    root = nc.dram_tensor("root", [128, 128], DT, kind="ExternalInput")
    brg = nc.dram_tensor("brg", [128, 1], F32, kind="ExternalInput")
    w1 = nc.dram_tensor("w1", [128, 128], DT, kind="ExternalInput")
    b1 = nc.dram_tensor("b1", [128, 1], F32, kind="ExternalInput")
    w2 = nc.dram_tensor("w2", [128, OUT], DT, kind="ExternalInput")
    b2 = nc.dram_tensor("b2", [OUT, 1], F32, kind="ExternalInput")
    outT = nc.dram_tensor("outT", [OUT, NP2], F32, kind="ExternalOutput")

    with tile.TileContext(nc) as tc:
        nc.gpsimd.load_library(library_config.mlp)
        with ExitStack() as stack:
            cpool = stack.enter_context(tc.tile_pool(name="const", bufs=1))
            dpool = stack.enter_context(
                tc.tile_pool(name="dram", bufs=1, space="DRAM"))
            persist = stack.enter_context(tc.tile_pool(name="persist", bufs=1))

            def cload(dram_t, shape, dtype):
                t = cpool.tile(shape, dtype, name=f"c_{dram_t.name}")
                nc.sync.dma_start(t[:], dram_t[:])
                return t

            wt_s = cload(wt, [128, KT, 128], DT)
            bt_s = cload(bt, [128, 1], F32)
            win_s = cload(win, [128, 128], DT)
            bin_s = cload(bin_, [128, 1], F32)
            wr_s = cload(wr, [128, R * 128], DT)
            root_s = cload(root, [128, 128], DT)
            brg_s = cload(brg, [128, 1], F32)
            w1_s = cload(w1, [128, 128], DT)
            b1_s = cload(b1, [128, 1], F32)
            w2_s = cload(w2, [128, OUT], DT)
            b2_s = cload(b2, [OUT, 1], F32)
            idx_s = cload(idx16, [128, TIDX // 16], mybir.dt.int16)
            slotw_s = cload(slotw, [128, TC, 4], F32)
            iota_s = cload(iota, [128, R * WIN], BF16)
            ident_s = cload(ident, [128, 128], DT)

            tables = [dpool.tile([C * NP2, 128], BF16, addr_space="Shared",
                                 name=f"table{i}") for i in range(2)]
            agin = dpool.tile([NP2, 128], BF16, name="agin")

            xT = persist.tile([128, NP2], DT, name="xT")

            gsems = [nc.alloc_semaphore(f"gsem{q}") for q in range(N_QUEUES)]
            pe_done = nc.alloc_semaphore("pe_done")
            for q in range(N_QUEUES):
                nc.gpsimd.sem_clear(gsems[q])
            nc.gpsimd.sem_clear(pe_done)
            gcount = [0] * N_QUEUES
            sg_done = [0]  # supergroups fully consumed by the PE so far

            # ---------------- stage 1: x = lrelu(lrelu(tweet@Wt+bt)@Win+bin)
            with tc.tile_pool(name="s1", bufs=2) as s1p, \
                 tc.tile_pool(name="s1t", bufs=4) as ttp, \
                 tc.tile_pool(name="ps1", bufs=4, space="PSUM") as ps1:
                for (c0, fw) in _col_tiles(NP2, FW0):
                    twt = s1p.tile([128, KT, fw], DT, tag="twt", name="twt")
                    nc.sync.dma_start(twt[:], twT[:, :, c0:c0 + fw])
                    for (c1, fw2) in _col_tiles(fw, 512):
                        ps_t = ps1.tile([128, fw2], F32, tag="pst", name="ps_t")
                        for k in range(KT):
                            nc.tensor.matmul(ps_t[:], wt_s[:, k, :],
                                             twt[:, k, c1:c1 + fw2],
                                             start=(k == 0), stop=(k == KT - 1))
                        tt = ttp.tile([128, fw2], DT, tag="tt", name="tt")
                        nc.scalar.activation(tt[:], ps_t[:], AF.Lrelu,
                                             bias=bt_s[:], alpha=0.01)
                        ps_x = ps1.tile([128, fw2], F32, tag="psx", name="ps_x")
                        nc.tensor.matmul(ps_x[:], win_s[:], tt[:],
                                         start=True, stop=True)
                        nc.scalar.activation(xT[:, c0 + c1:c0 + c1 + fw2],
                                             ps_x[:], AF.Lrelu,
                                             bias=bin_s[:], alpha=0.01)

            # ---------------- 2 RGCN layers
            for layer in range(2):
                table = tables[layer]
                # phase A: transpose x windows to node-major + AllGather
                with tc.tile_pool(name=f"pa{layer}", bufs=3) as pap, \
                     tc.tile_pool(name=f"psa{layer}", bufs=2,
                                  space="PSUM") as psa:
                    for nt in range(NW):
                        psT = psa.tile([128, 128], DT, tag="psT", name="psT")
                        nc.tensor.transpose(psT[:],
                                            xT[:, nt * 128:(nt + 1) * 128],
                                            ident_s[:])
                        ob = pap.tile([128, 128], BF16, tag="ob", name="ob")
                        nc.scalar.activation(ob[:], psT[:], AF.Copy)
                        nc.sync.dma_start(
                            agin[nt * 128:(nt + 1) * 128, :], ob[:])
                    nc.gpsimd.collective_compute(
                        "AllGather", mybir.AluOpType.bypass,
                        replica_groups=[list(range(C))],
                        ins=[agin[:]],
                        outs=[table[:]])
                    # Pool-queue probe read of the table: its wait on the
                    # collective blocks the Pool sequencer, so the deferred
                    # gather DMAs (triggered below) cannot start early.
                    probe = pap.tile([128, 2], BF16, tag="probe",
                                     name="probe")
                    probe_bi = nc.gpsimd.dma_start(probe[:], table[0:128, 0:2])

                # phase B: gather + on-device one-hot scatter matmuls
                per_sg = {}
                for (b, gc0, nch, s0) in gather_insts:
                    per_sg[s0] = per_sg.get(s0, 0) + 1
                g_bufs = G_BUFS or ((PIPE_DEPTH + 1) * max(per_sg.values()) + 4)
                with tc.tile_pool(name=f"g{layer}", bufs=g_bufs) as gp, \
                     tc.tile_pool(name=f"s{layer}", bufs=S_BUFS) as sp, \
                     tc.tile_pool(name=f"u{layer}", bufs=S_BUFS) as up, \
                     tc.tile_pool(name=f"m{layer}", bufs=M_BUFS) as mp, \
                     tc.tile_pool(name=f"pb{layer}", bufs=PS_BUFS,
                                  space="PSUM") as pb, \
                     tc.tile_pool(name=f"pc{layer}", bufs=PSB_BUFS,
                                  space="PSUM") as pc:
                    by_sg = {}
                    qload = [0] * N_QUEUES
                    for (b, gc0, nch, s0) in gather_insts:
                        qi = qload.index(min(qload))
                        qload[qi] += nch
                        by_sg.setdefault(s0, []).append((b, gc0, nch, qi))
                    sidx = 0
                    prev_triggers = []
                    prev_inc = None
                    for si, s0 in enumerate(range(0, NW, SG_WINDOWS)):
                        vs = range(s0, min(s0 + SG_WINDOWS, NW))
                        gts = {}
                        used_q = set()
                        gate = sg_done[0] + si - PIPE_DEPTH
                        gate_bi = None
                        if PREP_ONLY and gate > 0:
                            # don't run more than PIPE_DEPTH supergroups ahead
                            # of the PE: bounds gather-buffer reuse (WAR)
                            gate_bi = nc.gpsimd.wait_ge(pe_done, gate)
                            _order(gate_bi, prev_triggers)
                        for (b, gc0, nch, qi) in by_sg.get(s0, []):
                            gt = gp.tile([128, nch, 128], BF16, tag="g",
                                         name="gt")
                            if PREP_ONLY:
                                p_bi = nc.gpsimd.dma_gather(
                                    gt[:], table[b * NP2:(b + 1) * NP2, :],
                                    idx_s[:, gc0 * 8: (gc0 + nch) * 8],
                                    nch * WIN, nch * WIN, 128,
                                    single_packet=SINGLE_PACKET,
                                    prepare_only=True, sem=gsems[qi],
                                    queue_num=qi)
                                _order(p_bi, [probe_bi, gate_bi])
                                used_q.add(qi)
                                gcount[qi] += 1
                            else:
                                nc.gpsimd.dma_gather(
                                    gt[:], table[b * NP2:(b + 1) * NP2, :],
                                    idx_s[:, gc0 * 8: (gc0 + nch) * 8],
                                    nch * WIN, nch * WIN, 128,
                                    single_packet=SINGLE_PACKET,
                                    queue_num=qi)
                            gts.setdefault(b, []).append((gt, gc0, nch))
                        triggers = []
                        waits = []
                        for qi in sorted(used_q):
                            triggers.append(nc.gpsimd.trigger_dma(
                                count=None, queue_num=qi))
                        for qi in sorted(used_q):
                            # user-synced gate: PE proceeds only once every
                            # gather of this supergroup has landed in SBUF
                            w_bi = nc.tensor.wait_ge(gsems[qi],
                                                     16 * gcount[qi])
                            _order(w_bi, triggers + [prev_inc])
                            waits.append(w_bi)
                        sg_mms = []
                        for v in vs:
                            nch_v = int(K[v].sum())
                            psB = pc.tile([128, WIN], F32, tag="psb",
                                          name="psB")
                            if nch_v:
                                psA = pb.tile([128, R * WIN], F32, tag="psA",
                                              name="psA")
                                i = 0
                                for b in range(C):
                                    for k in range(int(K[v, b])):
                                        cg = int(go_base[v, b]) + k
                                        ccx = int(co_base[v, b]) + k
                                        gt = None
                                        for (g_t, g_0, g_n) in gts[b]:
                                            if g_0 <= cg < g_0 + g_n:
                                                gt, j = g_t, cg - g_0
                                                break
                                        st = sp.tile([128, R * WIN], BF16,
                                                     tag="S", name="st")
                                        if sidx % SCALAR_MOD != SCALAR_MOD - 1:
                                            nc.vector.tensor_scalar(
                                                st[:], iota_s[:],
                                                slotw_s[:, ccx, 0:1],
                                                slotw_s[:, ccx, 1:2],
                                                op0=ALU.is_equal, op1=ALU.mult)
                                        else:
                                            ut = up.tile([128, R * WIN], BF16,
                                                         tag="u", name="ut")
                                            nc.scalar.activation(
                                                ut[:], iota_s[:], AF.Abs,
                                                bias=slotw_s[:, ccx, 2:3])
                                            nc.scalar.activation(
                                                st[:], ut[:], AF.Relu,
                                                bias=slotw_s[:, ccx, 1:2],
                                                scale=slotw_s[:, ccx, 3:4])
                                        sidx += 1
                                        mm_bi = nc.tensor.matmul(
                                            psA[:], gt[:, j, :], st[:],
                                            start=(i == 0),
                                            stop=(i == nch_v - 1))
                                        if PREP_ONLY:
                                            _order(mm_bi, waits)
                                            sg_mms.append(mm_bi)
                                        i += 1
                                m = mp.tile([128, R * WIN], BF16, tag="m",
                                            name="m")
                                nc.scalar.activation(m[:], psA[:], AF.Copy)
                                nc.tensor.matmul(psB[:], wr_s[:, 0:128],
                                                 m[:, 0:128],
                                                 start=True, stop=False)
                                nc.tensor.matmul(psB[:], wr_s[:, 128:256],
                                                 m[:, 128:256],
                                                 start=False, stop=False,
                                                 skip_group_check=True)
                                nc.tensor.matmul(psB[:], root_s[:],
                                                 xT[:, v * 128:(v + 1) * 128],
                                                 start=False, stop=True,
                                                 skip_group_check=True)
                            else:
                                nc.tensor.matmul(psB[:], root_s[:],
                                                 xT[:, v * 128:(v + 1) * 128],
                                                 start=True, stop=True)
                            nc.vector.tensor_scalar(
                                xT[:, v * 128:(v + 1) * 128], psB[:],
                                brg_s[:], None, op0=ALU.add)
                        if PREP_ONLY:
                            inc_bi = nc.tensor.sem_inc(pe_done, 1)
                            _order(inc_bi, sg_mms + waits)
                            prev_inc = inc_bi
                            prev_triggers = triggers
                    sg_done[0] += len(range(0, NW, SG_WINDOWS))

            # ---------------- head
            with tc.tile_pool(name="hd", bufs=3) as hp, \
                 tc.tile_pool(name="psh", bufs=2, space="PSUM") as psh, \
                 tc.tile_pool(name="outp", bufs=1) as outp:
                outT_s = outp.tile([OUT, NP2], F32, name="outT_s")
                for (c0, fw) in _col_tiles(NP2, 512):
                    ph = psh.tile([128, fw], F32, tag="ph", name="ph")
                    nc.tensor.matmul(ph[:], w1_s[:], xT[:, c0:c0 + fw],
                                     start=True, stop=True)
                    ht = hp.tile([128, fw], DT, tag="ht", name="ht")
                    nc.scalar.activation(ht[:], ph[:], AF.Lrelu,
                                         bias=b1_s[:], alpha=0.01)
                    po = psh.tile([OUT, fw], F32, tag="po", name="po")
                    nc.tensor.matmul(po[:], w2_s[:], ht[:],
                                     start=True, stop=True)
                    nc.vector.tensor_scalar(outT_s[:, c0:c0 + fw], po[:],
                                            b2_s[:], None, op0=ALU.add)
                nc.sync.dma_start(outT[:, :], outT_s[:])

    nc.compile()
    return nc


# ----------------------------------------------------------------------------
# entry point
# ----------------------------------------------------------------------------

def kernel(**inputs):
    global LAST_RESULTS
    tweet = np.asarray(inputs["tweet"], np.float32)
    ei = np.asarray(inputs["edge_index"]).astype(np.int64)
    et = np.asarray(inputs["edge_type"]).astype(np.int64)
    W_tweet = np.asarray(inputs["W_tweet"], np.float32)
    b_tweet = np.asarray(inputs["b_tweet"], np.float32)
    W_in = np.asarray(inputs["W_in"], np.float32)
    b_in = np.asarray(inputs["b_in"], np.float32)
    rgcn_weight = np.asarray(inputs["rgcn_weight"], np.float32)
    rgcn_root = np.asarray(inputs["rgcn_root"], np.float32)
    rgcn_bias = np.asarray(inputs["rgcn_bias"], np.float32)
    W_out1 = np.asarray(inputs["W_out1"], np.float32)
    b_out1 = np.asarray(inputs["b_out1"], np.float32)
    W_out2 = np.asarray(inputs["W_out2"], np.float32)
    b_out2 = np.asarray(inputs["b_out2"], np.float32)

    N, TW = tweet.shape
    D = W_in.shape[0]
    OUT = W_out2.shape[1]
    assert N % C == 0 and TW % 128 == 0 and D == 128
    NPC = N // C
    NP2 = -(-NPC // WIN) * WIN
    src, dst = ei[0], ei[1]

    meta = _edge_meta(src, dst, et, N, NPC, NP2)
    shapes = dict(N=N, TW=TW, D=D, OUT=OUT, NPC=NPC, NP2=NP2)
    npdt = _np_dt()
    KT = TW // 128

    nc = _build_program(shapes, meta)

    shared = {
        "iota": np.tile(np.arange(R * WIN, dtype=np.float32)
                        .astype(ml_dtypes.bfloat16), (128, 1)),
        "ident": np.eye(128, dtype=np.float32).astype(npdt),
        "wt": np.ascontiguousarray(
            W_tweet.reshape(KT, 128, 128).transpose(1, 0, 2)).astype(npdt),
        "bt": b_tweet.reshape(128, 1),
        "win": W_in.astype(npdt),
        "bin": b_in.reshape(128, 1),
        "wr": np.ascontiguousarray(
            rgcn_weight.transpose(1, 0, 2).reshape(128, R * 128)).astype(npdt),
        "root": rgcn_root.astype(npdt),
        "brg": rgcn_bias.reshape(128, 1),
        "w1": W_out1.astype(npdt),
        "b1": b_out1.reshape(128, 1),
        "w2": W_out2.astype(npdt),
        "b2": b_out2.reshape(OUT, 1),
    }

    in_maps = []
    for c in range(C):
        tw_c = np.zeros((128, KT, NP2), npdt)
        tw_c[:, :, :NPC] = (tweet[c * NPC:(c + 1) * NPC].T
                            .reshape(KT, 128, NPC).transpose(1, 0, 2)
                            .astype(npdt))
        m = dict(shared)
        m["twT"] = tw_c
        m["idx16"] = meta["idxw"][c]
        m["slotw"] = meta["slotw"][c]
        in_maps.append(m)

    res = run_bass_kernel_spmd(nc, in_maps, core_ids=list(range(C)),
                               trace=TRACE, tmpdir=TMPDIR)
    LAST_RESULTS = res

    out = np.zeros((N, OUT), np.float32)
    for c in range(C):
        out[c * NPC:(c + 1) * NPC] = res.results[c]["outT"][:, :NPC].T
    return out


# revision 21
# speedup vs baseline: 1.0041x; 1.0041x over previous
"""Trainium2 Bass kernel for nn_BotRGCN2 (2-layer RGCN over 100k nodes / 600k edges).

Strategy (8 NeuronCores, SPMD):
  - Shard nodes across cores (12500/core, padded to 12544 = 98 windows of 128).
  - Feature-major (transposed) activations on-chip; node-major gather table in
    DRAM.
  - Gather-first RGCN: AllGather the raw x shards (node-major, bf16) into a
    full [C*NP2, 128] table per layer; per owned 128-node window, dma_gather
    the per-edge source rows and scatter-add them on the PE:
    psum[feat, slot] += G^T @ S with lhsT = G (gathered rows
    [128 edges x 128 feat]) and rhs = S ([128 edges x 256 slots], slot =
    rel*128 + dst_local, value 1/cnt(dst,rel)).  S is built ON DEVICE by the
    DVE from a 4-byte/edge (slot, weight) stream: S = (iota == slot) * w.
    Per-relation weights are applied AFTER aggregation (mean is linear):
    x_new = sum_r W_r^T @ mean_r + root^T @ x + bias  (3 matmuls/window).
  - Edges preprocessed on host: partitioned by dst owner, grouped by
    (window, src-owner-block), padded to 128-edge chunks with weight-0
    entries; chunk structure shared by all 8 cores (max over cores) so one
    SPMD program serves every core.  Gathers capped at 1024 indices and
    round-robined over 4 SWDGE queues.  Index and slot/weight streams are
    identical for both layers and loaded once.
"""

import sys
from contextlib import ExitStack

import numpy as np

sys.path.insert(0, "/opt/trn_rl_repo")

import ml_dtypes  # noqa: E402
import concourse.bass as bass  # noqa: E402,F401
import concourse.bacc as bacc  # noqa: E402
import concourse.mybir as mybir  # noqa: E402
import concourse.tile as tile  # noqa: E402
from concourse import library_config  # noqa: E402
from concourse.bass_utils import run_bass_kernel_spmd  # noqa: E402
from concourse.instruction_name_ordered_set import (  # noqa: E402
    InstructionNameOrderedSet,
)


def _order(after_bi, befores):
    """Anchor `after_bi` behind `befores` with no-sync (ordering-only) edges
    so the Tile scheduler cannot reorder user-synced semaphore protocol."""
    deps = InstructionNameOrderedSet()
    for b in befores:
        if b is not None:
            deps.add(b.ins.name)
    if deps:
        after_bi.ins.add_nosync_dependencies_from(deps)

C = 8           # cores
WIN = 128       # dst nodes per PSUM window
R = 2           # relations

# tunables
SG_WINDOWS = 8       # windows per gather supergroup
NIDX_CHUNKS_MAX = 8  # max 128-idx chunks per dma_gather (carveout limit)
SINGLE_PACKET = False
PREP_ONLY = True    # prepare_only + trigger_dma path
PIPE_DEPTH = 1      # supergroups the Pool queue may run ahead of the PE
SCALAR_MOD = 5      # 1 in SCALAR_MOD S-builds go to the scalar engine
G_BUFS = 0           # 0 = auto (gather insts per supergroup + headroom)
S_BUFS = 32          # on-device one-hot S tiles in flight
PS_BUFS = 4
PSB_BUFS = 3
M_BUFS = 4
N_QUEUES = 4         # SWDGE queues; gathers round-robin across them
FW0 = 4096           # stage-1 DMA tile width (8KB descriptors)
USE_BF16 = True      # bf16 activations (messages are always bf16)
TRACE = False
TMPDIR = None

F32 = mybir.dt.float32
BF16 = mybir.dt.bfloat16
LAST_RESULTS = None  # BassKernelResults of the most recent run


def _dt():
    return BF16 if USE_BF16 else F32


def _np_dt():
    return ml_dtypes.bfloat16 if USE_BF16 else np.float32


def _col_tiles(total, width):
    out = []
    c = 0
    while c < total:
        out.append((c, min(width, total - c)))
        c += width
    return out


# ----------------------------------------------------------------------------
# host-side edge preprocessing
# ----------------------------------------------------------------------------

def _edge_meta(src, dst, et, N, NPC, NP2):
    """Build the SPMD-uniform chunk structure, per-core gather index streams,
    and per-core (slot, weight) streams for on-device one-hot S tiles."""
    E = src.shape[0]
    NW = NP2 // WIN
    seg = dst * R + et
    cnt = np.bincount(seg, minlength=N * R).astype(np.float64)
    w = (1.0 / np.maximum(cnt, 1.0))[seg]

    core = dst // NPC
    nl = dst % NPC
    vwin = nl // WIN
    dloc = nl % WIN
    blk = src // NPC                       # table block == src owner core
    tloc = src % NPC                       # row within block (< NP2 <= 32767)
    slot = et * WIN + dloc                 # 0..2*WIN-1
    assert NP2 <= 32768

    counts = np.zeros((C, NW, C), np.int64)
    np.add.at(counts, (core, vwin, blk), 1)
    K = -(-counts.max(axis=0) // WIN)      # [NW, C] chunks per (win, blk)

    # compute-order chunk bases (v-major, then b, then k)
    co_base = np.zeros((NW, C), np.int64)
    cc = 0
    for v in range(NW):
        for b in range(C):
            co_base[v, b] = cc
            cc += K[v, b]
    TC = int(cc)

    # gather-order (supergroup, block, window, k) + gather instruction list
    go_base = np.zeros((NW, C), np.int64)
    gather_insts = []  # (blk, start_chunk, n_chunks, sg_start)
    gc = 0
    for s0 in range(0, NW, SG_WINDOWS):
        vs = range(s0, min(s0 + SG_WINDOWS, NW))
        for b in range(C):
            nch = int(sum(int(K[v, b]) for v in vs))
            if nch == 0:
                continue
            off = 0
            while off < nch:
                n = min(NIDX_CHUNKS_MAX, nch - off)
                gather_insts.append((b, gc + off, n, s0))
                off += n
            for v in vs:
                go_base[v, b] = gc
                gc += K[v, b]
    assert gc == TC

    # per-core streams
    order = np.argsort((core * NW + vwin) * C + blk, kind="stable")
    gid = ((core * NW + vwin) * C + blk)[order]
    starts = np.concatenate([[0], np.cumsum(np.bincount(gid, minlength=C * NW * C))])
    rank = np.arange(E) - starts[gid]

    ce = core[order]
    v_ = vwin[order]
    b_ = blk[order]
    k_ = rank // WIN
    lane = rank % WIN

    idxg = np.zeros((C, TC * WIN), np.int16)
    gpos = (go_base[v_, b_] + k_) * WIN + lane
    idxg[ce, gpos] = tloc[order].astype(np.int16)

    # per-edge (slot, w, -slot, -w) in compute order: lane-major [C, WIN, TC, 4]
    slotw = np.zeros((C, WIN, TC, 4), np.float32)
    cchunk = co_base[v_, b_] + k_
    slotw[ce, lane, cchunk, 0] = slot[order]
    slotw[ce, lane, cchunk, 1] = w[order].astype(np.float32)
    slotw[ce, lane, cchunk, 2] = -slot[order]
    slotw[ce, lane, cchunk, 3] = -w[order].astype(np.float32)

    # wrap indices per gather instruction: idx i -> [i%16, off + i//16]
    TIDX = TC * WIN
    idxw = np.zeros((C, 128, TIDX // 16), np.int16)
    for (b, gc0, nch, s0) in gather_insts:
        n = nch * WIN
        segm = idxg[:, gc0 * WIN: gc0 * WIN + n].reshape(C, n // 16, 16)
        idxw[:, :16, gc0 * 8: gc0 * 8 + n // 16] = segm.transpose(0, 2, 1)
    idxw[:, 16:, :] = np.tile(idxw[:, :16, :], (1, 7, 1))

    return dict(K=K, co_base=co_base, go_base=go_base,
                gather_insts=gather_insts,
                TC=TC, TIDX=TIDX, NW=NW, idxw=idxw, slotw=slotw)


# ----------------------------------------------------------------------------
# device program
# ----------------------------------------------------------------------------

def _build_program(shapes, meta):
    DT = _dt()
    N, TW, D, OUT, NPC, NP2 = (shapes[k] for k in
                               ("N", "TW", "D", "OUT", "NPC", "NP2"))
    KT = TW // 128
    NW = meta["NW"]
    TC, TIDX = meta["TC"], meta["TIDX"]
    K, co_base, go_base = meta["K"], meta["co_base"], meta["go_base"]
    gather_insts = meta["gather_insts"]
    AF = mybir.ActivationFunctionType
    ALU = mybir.AluOpType

    nc = bacc.Bacc("TRN2", target_bir_lowering=False,
                   num_swdge_queues=N_QUEUES)

    twT = nc.dram_tensor("twT", [128, KT, NP2], DT, kind="ExternalInput")
    idx16 = nc.dram_tensor("idx16", [128, TIDX // 16], mybir.dt.int16,
                           kind="ExternalInput")
    slotw = nc.dram_tensor("slotw", [128, TC, 4], F32, kind="ExternalInput")
    iota = nc.dram_tensor("iota", [128, R * WIN], BF16, kind="ExternalInput")
    ident = nc.dram_tensor("ident", [128, 128], DT, kind="ExternalInput")
    wt = nc.dram_tensor("wt", [128, KT, 128], DT, kind="ExternalInput")
    bt = nc.dram_tensor("bt", [128, 1], F32, kind="ExternalInput")
    win = nc.dram_tensor("win", [128, 128], DT, kind="ExternalInput")
    bin_ = nc.dram_tensor("bin", [128, 1], F32, kind="ExternalInput")
    wr = nc.dram_tensor("wr", [128, R * 128], DT, kind="ExternalInput")
    root = nc.dram_tensor("root", [128, 128], DT, kind="ExternalInput")
    brg = nc.dram_tensor("brg", [128, 1], F32, kind="ExternalInput")
    w1 = nc.dram_tensor("w1", [128, 128], DT, kind="ExternalInput")
    b1 = nc.dram_tensor("b1", [128, 1], F32, kind="ExternalInput")
    w2 = nc.dram_tensor("w2", [128, OUT], DT, kind="ExternalInput")
    b2 = nc.dram_tensor("b2", [OUT, 1], F32, kind="ExternalInput")
    outT = nc.dram_tensor("outT", [OUT, NP2], F32, kind="ExternalOutput")

    with tile.TileContext(nc) as tc:
        nc.gpsimd.load_library(library_config.mlp)
        with ExitStack() as stack:
            cpool = stack.enter_context(tc.tile_pool(name="const", bufs=1))
            dpool = stack.enter_context(
                tc.tile_pool(name="dram", bufs=1, space="DRAM"))
            persist = stack.enter_context(tc.tile_pool(name="persist", bufs=1))

            def cload(dram_t, shape, dtype):
                t = cpool.tile(shape, dtype, name=f"c_{dram_t.name}")
                nc.sync.dma_start(t[:], dram_t[:])
                return t

            wt_s = cload(wt, [128, KT, 128], DT)
            bt_s = cload(bt, [128, 1], F32)
            win_s = cload(win, [128, 128], DT)
            bin_s = cload(bin_, [128, 1], F32)
            wr_s = cload(wr, [128, R * 128], DT)
            root_s = cload(root, [128, 128], DT)
            brg_s = cload(brg, [128, 1], F32)
            w1_s = cload(w1, [128, 128], DT)
            b1_s = cload(b1, [128, 1], F32)
            w2_s = cload(w2, [128, OUT], DT)
            b2_s = cload(b2, [OUT, 1], F32)
            idx_s = cload(idx16, [128, TIDX // 16], mybir.dt.int16)
            slotw_s = cload(slotw, [128, TC, 4], F32)
            iota_s = cload(iota, [128, R * WIN], BF16)
            ident_s = cload(ident, [128, 128], DT)

            tables = [dpool.tile([C * NP2, 128], BF16, addr_space="Shared",
                                 name=f"table{i}") for i in range(2)]
            agin = dpool.tile([NP2, 128], BF16, name="agin")

            xT = persist.tile([128, NP2], DT, name="xT")

            gsems = [nc.alloc_semaphore(f"gsem{q}") for q in range(N_QUEUES)]
            pe_done = nc.alloc_semaphore("pe_done")
            for q in range(N_QUEUES):
                nc.gpsimd.sem_clear(gsems[q])
            nc.gpsimd.sem_clear(pe_done)
            gcount = [0] * N_QUEUES
            sg_done = [0]  # supergroups fully consumed by the PE so far

            # ---------------- stage 1: x = lrelu(lrelu(tweet@Wt+bt)@Win+bin)
            with tc.tile_pool(name="s1", bufs=2) as s1p, \
                 tc.tile_pool(name="s1t", bufs=4) as ttp, \
                 tc.tile_pool(name="ps1", bufs=4, space="PSUM") as ps1:
                for (c0, fw) in _col_tiles(NP2, FW0):
                    twt = s1p.tile([128, KT, fw], DT, tag="twt", name="twt")
                    nc.sync.dma_start(twt[:], twT[:, :, c0:c0 + fw])
                    for (c1, fw2) in _col_tiles(fw, 512):
                        ps_t = ps1.tile([128, fw2], F32, tag="pst", name="ps_t")
                        for k in range(KT):
                            nc.tensor.matmul(ps_t[:], wt_s[:, k, :],
                                             twt[:, k, c1:c1 + fw2],
                                             start=(k == 0), stop=(k == KT - 1))
                        tt = ttp.tile([128, fw2], DT, tag="tt", name="tt")
                        nc.scalar.activation(tt[:], ps_t[:], AF.Lrelu,
                                             bias=bt_s[:], alpha=0.01)
                        ps_x = ps1.tile([128, fw2], F32, tag="psx", name="ps_x")
                        nc.tensor.matmul(ps_x[:], win_s[:], tt[:],
                                         start=True, stop=True)
                        nc.scalar.activation(xT[:, c0 + c1:c0 + c1 + fw2],
                                             ps_x[:], AF.Lrelu,
                                             bias=bin_s[:], alpha=0.01)

            # ---------------- 2 RGCN layers
            for layer in range(2):
                table = tables[layer]
                # phase A: transpose x windows to node-major + AllGather
                with tc.tile_pool(name=f"pa{layer}", bufs=3) as pap, \
                     tc.tile_pool(name=f"psa{layer}", bufs=2,
                                  space="PSUM") as psa:
                    for nt in range(NW):
                        psT = psa.tile([128, 128], DT, tag="psT", name="psT")
                        nc.tensor.transpose(psT[:],
                                            xT[:, nt * 128:(nt + 1) * 128],
                                            ident_s[:])
                        ob = pap.tile([128, 128], BF16, tag="ob", name="ob")
                        nc.scalar.activation(ob[:], psT[:], AF.Copy)
                        nc.sync.dma_start(
                            agin[nt * 128:(nt + 1) * 128, :], ob[:])
                    nc.gpsimd.collective_compute(
                        "AllGather", mybir.AluOpType.bypass,
                        replica_groups=[list(range(C))],
                        ins=[agin[:]],
                        outs=[table[:]])
                    # Pool-queue probe read of the table: its wait on the
                    # collective blocks the Pool sequencer, so the deferred
                    # gather DMAs (triggered below) cannot start early.
                    probe = pap.tile([128, 2], BF16, tag="probe",
                                     name="probe")
                    probe_bi = nc.gpsimd.dma_start(probe[:], table[0:128, 0:2])

                # phase B: gather + on-device one-hot scatter matmuls
                per_sg = {}
                for (b, gc0, nch, s0) in gather_insts:
                    per_sg[s0] = per_sg.get(s0, 0) + 1
                g_bufs = G_BUFS or ((PIPE_DEPTH + 1) * max(per_sg.values()) + 4)
                with tc.tile_pool(name=f"g{layer}", bufs=g_bufs) as gp, \
                     tc.tile_pool(name=f"s{layer}", bufs=S_BUFS) as sp, \
                     tc.tile_pool(name=f"u{layer}", bufs=S_BUFS) as up, \
                     tc.tile_pool(name=f"m{layer}", bufs=M_BUFS) as mp, \
                     tc.tile_pool(name=f"pb{layer}", bufs=PS_BUFS,
                                  space="PSUM") as pb, \
                     tc.tile_pool(name=f"pc{layer}", bufs=PSB_BUFS,
                                  space="PSUM") as pc:
                    by_sg = {}
                    qload = [0] * N_QUEUES
                    for (b, gc0, nch, s0) in gather_insts:
                        qi = qload.index(min(qload))
                        qload[qi] += nch
                        by_sg.setdefault(s0, []).append((b, gc0, nch, qi))
                    sidx = 0
                    prev_triggers = []
                    prev_inc = None
                    for si, s0 in enumerate(range(0, NW, SG_WINDOWS)):
                        vs = range(s0, min(s0 + SG_WINDOWS, NW))
                        gts = {}
                        used_q = set()
                        gate = sg_done[0] + si - PIPE_DEPTH
                        gate_bi = None
                        if PREP_ONLY and gate > 0:
                            # don't run more than PIPE_DEPTH supergroups ahead
                            # of the PE: bounds gather-buffer reuse (WAR)
                            gate_bi = nc.gpsimd.wait_ge(pe_done, gate)
                            _order(gate_bi, prev_triggers)
                        for (b, gc0, nch, qi) in by_sg.get(s0, []):
                            gt = gp.tile([128, nch, 128], BF16, tag="g",
                                         name="gt")
                            if PREP_ONLY:
                                p_bi = nc.gpsimd.dma_gather(
                                    gt[:], table[b * NP2:(b + 1) * NP2, :],
                                    idx_s[:, gc0 * 8: (gc0 + nch) * 8],
                                    nch * WIN, nch * WIN, 128,
                                    single_packet=SINGLE_PACKET,
                                    prepare_only=True, sem=gsems[qi],
                                    queue_num=qi)
                                _order(p_bi, [probe_bi, gate_bi])
                                used_q.add(qi)
                                gcount[qi] += 1
                            else:
                                nc.gpsimd.dma_gather(
                                    gt[:], table[b * NP2:(b + 1) * NP2, :],
                                    idx_s[:, gc0 * 8: (gc0 + nch) * 8],
                                    nch * WIN, nch * WIN, 128,
                                    single_packet=SINGLE_PACKET,
                                    queue_num=qi)
                            gts.setdefault(b, []).append((gt, gc0, nch))
                        triggers = []
                        waits = []
                        for qi in sorted(used_q):
                            triggers.append(nc.gpsimd.trigger_dma(
                                count=None, queue_num=qi))
                        for qi in sorted(used_q):
                            # user-synced gate: PE proceeds only once every
                            # gather of this supergroup has landed in SBUF
                            w_bi = nc.tensor.wait_ge(gsems[qi],
                                                     16 * gcount[qi])
                            _order(w_bi, triggers + [prev_inc])
                            waits.append(w_bi)
                        sg_mms = []
                        for v in vs:
                            nch_v = int(K[v].sum())
                            psB = pc.tile([128, WIN], F32, tag="psb",
                                          name="psB")
                            if nch_v:
                                psA = pb.tile([128, R * WIN], F32, tag="psA",
                                              name="psA")
                                i = 0
                                for b in range(C):
                                    for k in range(int(K[v, b])):
                                        cg = int(go_base[v, b]) + k
                                        ccx = int(co_base[v, b]) + k
                                        gt = None
                                        for (g_t, g_0, g_n) in gts[b]:
                                            if g_0 <= cg < g_0 + g_n:
                                                gt, j = g_t, cg - g_0
                                                break
                                        st = sp.tile([128, R * WIN], BF16,
                                                     tag="S", name="st")
                                        if sidx % SCALAR_MOD != SCALAR_MOD - 1:
                                            nc.vector.tensor_scalar(
                                                st[:], iota_s[:],
                                                slotw_s[:, ccx, 0:1],
                                                slotw_s[:, ccx, 1:2],
                                                op0=ALU.is_equal, op1=ALU.mult)
                                        else:
                                            ut = up.tile([128, R * WIN], BF16,
                                                         tag="u", name="ut")
                                            nc.scalar.activation(
                                                ut[:], iota_s[:], AF.Abs,
                                                bias=slotw_s[:, ccx, 2:3])
                                            nc.scalar.activation(
                                                st[:], ut[:], AF.Relu,
                                                bias=slotw_s[:, ccx, 1:2],
                                                scale=slotw_s[:, ccx, 3:4])
                                        sidx += 1
                                        mm_bi = nc.tensor.matmul(
                                            psA[:], gt[:, j, :], st[:],
                                            start=(i == 0),
                                            stop=(i == nch_v - 1))
                                        if PREP_ONLY:
                                            _order(mm_bi, waits)
                                            sg_mms.append(mm_bi)
                                        i += 1
                                m = mp.tile([128, R * WIN], BF16, tag="m",
                                            name="m")
                                nc.scalar.activation(m[:], psA[:], AF.Copy)
                                nc.tensor.matmul(psB[:], wr_s[:, 0:128],
                                                 m[:, 0:128],
                                                 start=True, stop=False)
                                nc.tensor.matmul(psB[:], wr_s[:, 128:256],
                                                 m[:, 128:256],
                                                 start=False, stop=False,
                                                 skip_group_check=True)
                                nc.tensor.matmul(psB[:], root_s[:],
                                                 xT[:, v * 128:(v + 1) * 128],
                                                 start=False, stop=True,
                                                 skip_group_check=True)
                            else:
                                nc.tensor.matmul(psB[:], root_s[:],
                                                 xT[:, v * 128:(v + 1) * 128],
                                                 start=True, stop=True)
                            nc.vector.tensor_scalar(
                                xT[:, v * 128:(v + 1) * 128], psB[:],
                                brg_s[:], None, op0=ALU.add)
                        if PREP_ONLY:
                            inc_bi = nc.tensor.sem_inc(pe_done, 1)
                            _order(inc_bi, sg_mms + waits)
                            prev_inc = inc_bi
                            prev_triggers = triggers
                    sg_done[0] += len(range(0, NW, SG_WINDOWS))

            # ---------------- head
            with tc.tile_pool(name="hd", bufs=3) as hp, \
                 tc.tile_pool(name="psh", bufs=2, space="PSUM") as psh, \
                 tc.tile_pool(name="outp", bufs=1) as outp:
                outT_s = outp.tile([OUT, NP2], F32, name="outT_s")
                for (c0, fw) in _col_tiles(NP2, 512):
                    ph = psh.tile([128, fw], F32, tag="ph", name="ph")
                    nc.tensor.matmul(ph[:], w1_s[:], xT[:, c0:c0 + fw],
                                     start=True, stop=True)
                    ht = hp.tile([128, fw], DT, tag="ht", name="ht")
                    nc.scalar.activation(ht[:], ph[:], AF.Lrelu,
                                         bias=b1_s[:], alpha=0.01)
                    po = psh.tile([OUT, fw], F32, tag="po", name="po")
                    nc.tensor.matmul(po[:], w2_s[:], ht[:],
                                     start=True, stop=True)
                    nc.vector.tensor_scalar(outT_s[:, c0:c0 + fw], po[:],
                                            b2_s[:], None, op0=ALU.add)
                nc.sync.dma_start(outT[:, :], outT_s[:])

    nc.compile()
    return nc


# ----------------------------------------------------------------------------
# entry point
# ----------------------------------------------------------------------------

def kernel(**inputs):
    global LAST_RESULTS
    tweet = np.asarray(inputs["tweet"], np.float32)
    ei = np.asarray(inputs["edge_index"]).astype(np.int64)
    et = np.asarray(inputs["edge_type"]).astype(np.int64)
    W_tweet = np.asarray(inputs["W_tweet"], np.float32)
    b_tweet = np.asarray(inputs["b_tweet"], np.float32)
    W_in = np.asarray(inputs["W_in"], np.float32)
    b_in = np.asarray(inputs["b_in"], np.float32)
    rgcn_weight = np.asarray(inputs["rgcn_weight"], np.float32)
    rgcn_root = np.asarray(inputs["rgcn_root"], np.float32)
    rgcn_bias = np.asarray(inputs["rgcn_bias"], np.float32)
    W_out1 = np.asarray(inputs["W_out1"], np.float32)
    b_out1 = np.asarray(inputs["b_out1"], np.float32)
    W_out2 = np.asarray(inputs["W_out2"], np.float32)
    b_out2 = np.asarray(inputs["b_out2"], np.float32)

    N, TW = tweet.shape
    D = W_in.shape[0]
    OUT = W_out2.shape[1]
    assert N % C == 0 and TW % 128 == 0 and D == 128
    NPC = N // C
    NP2 = -(-NPC // WIN) * WIN
    src, dst = ei[0], ei[1]

    meta = _edge_meta(src, dst, et, N, NPC, NP2)
    shapes = dict(N=N, TW=TW, D=D, OUT=OUT, NPC=NPC, NP2=NP2)
    npdt = _np_dt()
    KT = TW // 128

    nc = _build_program(shapes, meta)

    shared = {
        "iota": np.tile(np.arange(R * WIN, dtype=np.float32)
                        .astype(ml_dtypes.bfloat16), (128, 1)),
        "ident": np.eye(128, dtype=np.float32).astype(npdt),
        "wt": np.ascontiguousarray(
            W_tweet.reshape(KT, 128, 128).transpose(1, 0, 2)).astype(npdt),
        "bt": b_tweet.reshape(128, 1),
        "win": W_in.astype(npdt),
        "bin": b_in.reshape(128, 1),
        "wr": np.ascontiguousarray(
            rgcn_weight.transpose(1, 0, 2).reshape(128, R * 128)).astype(npdt),
        "root": rgcn_root.astype(npdt),
        "brg": rgcn_bias.reshape(128, 1),
        "w1": W_out1.astype(npdt),
        "b1": b_out1.reshape(128, 1),
        "w2": W_out2.astype(npdt),
        "b2": b_out2.reshape(OUT, 1),
    }

    in_maps = []
    for c in range(C):
        tw_c = np.zeros((128, KT, NP2), npdt)
        tw_c[:, :, :NPC] = (tweet[c * NPC:(c + 1) * NPC].T
                            .reshape(KT, 128, NPC).transpose(1, 0, 2)
                            .astype(npdt))
        m = dict(shared)
        m["twT"] = tw_c
        m["idx16"] = meta["idxw"][c]
        m["slotw"] = meta["slotw"][c]
        in_maps.append(m)

    res = run_bass_kernel_spmd(nc, in_maps, core_ids=list(range(C)),
                               trace=TRACE, tmpdir=TMPDIR)
    LAST_RESULTS = res

    out = np.zeros((N, OUT), np.float32)
    for c in range(C):
        out[c * NPC:(c + 1) * NPC] = res.results[c]["outT"][:, :NPC].T
    return out


# revision 22
# speedup vs baseline: 1.0365x; 1.0323x over previous
"""Trainium2 Bass kernel for nn_BotRGCN2 (2-layer RGCN over 100k nodes / 600k edges).

Strategy (8 NeuronCores, SPMD):
  - Shard nodes across cores (12500/core, padded to 12544 = 98 windows of 128).
  - Feature-major (transposed) activations on-chip; node-major gather table in
    DRAM.
  - Gather-first RGCN: AllGather the raw x shards (node-major, bf16) into a
    full [C*NP2, 128] table per layer; per owned 128-node window, dma_gather
    the per-edge source rows and scatter-add them on the PE:
    psum[feat, slot] += G^T @ S with lhsT = G (gathered rows
    [128 edges x 128 feat]) and rhs = S ([128 edges x 256 slots], slot =
    rel*128 + dst_local, value 1/cnt(dst,rel)).  S is built ON DEVICE by the
    DVE from a 4-byte/edge (slot, weight) stream: S = (iota == slot) * w.
    Per-relation weights are applied AFTER aggregation (mean is linear):
    x_new = sum_r W_r^T @ mean_r + root^T @ x + bias  (3 matmuls/window).
  - Edges preprocessed on host: partitioned by dst owner, grouped by
    (window, src-owner-block), padded to 128-edge chunks with weight-0
    entries; chunk structure shared by all 8 cores (max over cores) so one
    SPMD program serves every core.  Gathers capped at 1024 indices and
    round-robined over 4 SWDGE queues.  Index and slot/weight streams are
    identical for both layers and loaded once.
"""

import sys
from contextlib import ExitStack

import numpy as np

sys.path.insert(0, "/opt/trn_rl_repo")

import ml_dtypes  # noqa: E402
import concourse.bass as bass  # noqa: E402,F401
import concourse.bacc as bacc  # noqa: E402
import concourse.mybir as mybir  # noqa: E402
import concourse.tile as tile  # noqa: E402
from concourse import library_config  # noqa: E402
from concourse.bass_utils import run_bass_kernel_spmd  # noqa: E402
from concourse.instruction_name_ordered_set import (  # noqa: E402
    InstructionNameOrderedSet,
)


def _order(after_bi, befores):
    """Anchor `after_bi` behind `befores` with no-sync (ordering-only) edges
    so the Tile scheduler cannot reorder user-synced semaphore protocol."""
    deps = InstructionNameOrderedSet()
    for b in befores:
        if b is not None:
            deps.add(b.ins.name)
    if deps:
        after_bi.ins.add_nosync_dependencies_from(deps)

C = 8           # cores
WIN = 128       # dst nodes per PSUM window
R = 2           # relations

# tunables
SG_WINDOWS = 7       # windows per gather supergroup
NIDX_CHUNKS_MAX = 8  # max 128-idx chunks per dma_gather (carveout limit)
SINGLE_PACKET = False
PREP_ONLY = True    # prepare_only + trigger_dma path
PIPE_DEPTH = 2      # supergroups the Pool queue may run ahead of the PE
SCALAR_MOD = 5      # 1 in SCALAR_MOD S-builds go to the scalar engine
G_BUFS = 0           # 0 = auto (gather insts per supergroup + headroom)
S_BUFS = 32          # on-device one-hot S tiles in flight
PS_BUFS = 4
PSB_BUFS = 3
M_BUFS = 4
N_QUEUES = 4         # SWDGE queues; gathers round-robin across them
FW0 = 4096           # stage-1 DMA tile width (8KB descriptors)
USE_BF16 = True      # bf16 activations (messages are always bf16)
TRACE = False
TMPDIR = None

F32 = mybir.dt.float32
BF16 = mybir.dt.bfloat16
LAST_RESULTS = None  # BassKernelResults of the most recent run


def _dt():
    return BF16 if USE_BF16 else F32


def _np_dt():
    return ml_dtypes.bfloat16 if USE_BF16 else np.float32


def _col_tiles(total, width):
    out = []
    c = 0
    while c < total:
        out.append((c, min(width, total - c)))
        c += width
    return out


# ----------------------------------------------------------------------------
# host-side edge preprocessing
# ----------------------------------------------------------------------------

def _edge_meta(src, dst, et, N, NPC, NP2):
    """Build the SPMD-uniform chunk structure, per-core gather index streams,
    and per-core (slot, weight) streams for on-device one-hot S tiles."""
    E = src.shape[0]
    NW = NP2 // WIN
    seg = dst * R + et
    cnt = np.bincount(seg, minlength=N * R).astype(np.float64)
    w = (1.0 / np.maximum(cnt, 1.0))[seg]

    core = dst // NPC
    nl = dst % NPC
    vwin = nl // WIN
    dloc = nl % WIN
    blk = src // NPC                       # table block == src owner core
    tloc = src % NPC                       # row within block (< NP2 <= 32767)
    slot = et * WIN + dloc                 # 0..2*WIN-1
    assert NP2 <= 32768

    counts = np.zeros((C, NW, C), np.int64)
    np.add.at(counts, (core, vwin, blk), 1)
    K = -(-counts.max(axis=0) // WIN)      # [NW, C] chunks per (win, blk)

    # compute-order chunk bases (v-major, then b, then k)
    co_base = np.zeros((NW, C), np.int64)
    cc = 0
    for v in range(NW):
        for b in range(C):
            co_base[v, b] = cc
            cc += K[v, b]
    TC = int(cc)

    # gather-order (supergroup, block, window, k) + gather instruction list
    go_base = np.zeros((NW, C), np.int64)
    gather_insts = []  # (blk, start_chunk, n_chunks, sg_start)
    gc = 0
    for s0 in range(0, NW, SG_WINDOWS):
        vs = range(s0, min(s0 + SG_WINDOWS, NW))
        for b in range(C):
            nch = int(sum(int(K[v, b]) for v in vs))
            if nch == 0:
                continue
            off = 0
            while off < nch:
                n = min(NIDX_CHUNKS_MAX, nch - off)
                gather_insts.append((b, gc + off, n, s0))
                off += n
            for v in vs:
                go_base[v, b] = gc
                gc += K[v, b]
    assert gc == TC

    # per-core streams
    order = np.argsort((core * NW + vwin) * C + blk, kind="stable")
    gid = ((core * NW + vwin) * C + blk)[order]
    starts = np.concatenate([[0], np.cumsum(np.bincount(gid, minlength=C * NW * C))])
    rank = np.arange(E) - starts[gid]

    ce = core[order]
    v_ = vwin[order]
    b_ = blk[order]
    k_ = rank // WIN
    lane = rank % WIN

    idxg = np.zeros((C, TC * WIN), np.int16)
    gpos = (go_base[v_, b_] + k_) * WIN + lane
    idxg[ce, gpos] = tloc[order].astype(np.int16)

    # per-edge (slot, w, -slot, -w) in compute order: lane-major [C, WIN, TC, 4]
    slotw = np.zeros((C, WIN, TC, 4), np.float32)
    cchunk = co_base[v_, b_] + k_
    slotw[ce, lane, cchunk, 0] = slot[order]
    slotw[ce, lane, cchunk, 1] = w[order].astype(np.float32)
    slotw[ce, lane, cchunk, 2] = -slot[order]
    slotw[ce, lane, cchunk, 3] = -w[order].astype(np.float32)

    # wrap indices per gather instruction: idx i -> [i%16, off + i//16]
    TIDX = TC * WIN
    idxw = np.zeros((C, 128, TIDX // 16), np.int16)
    for (b, gc0, nch, s0) in gather_insts:
        n = nch * WIN
        segm = idxg[:, gc0 * WIN: gc0 * WIN + n].reshape(C, n // 16, 16)
        idxw[:, :16, gc0 * 8: gc0 * 8 + n // 16] = segm.transpose(0, 2, 1)
    idxw[:, 16:, :] = np.tile(idxw[:, :16, :], (1, 7, 1))

    return dict(K=K, co_base=co_base, go_base=go_base,
                gather_insts=gather_insts,
                TC=TC, TIDX=TIDX, NW=NW, idxw=idxw, slotw=slotw)


# ----------------------------------------------------------------------------
# device program
# ----------------------------------------------------------------------------

def _build_program(shapes, meta):
    DT = _dt()
    N, TW, D, OUT, NPC, NP2 = (shapes[k] for k in
                               ("N", "TW", "D", "OUT", "NPC", "NP2"))
    KT = TW // 128
    NW = meta["NW"]
    TC, TIDX = meta["TC"], meta["TIDX"]
    K, co_base, go_base = meta["K"], meta["co_base"], meta["go_base"]
    gather_insts = meta["gather_insts"]
    AF = mybir.ActivationFunctionType
    ALU = mybir.AluOpType

    nc = bacc.Bacc("TRN2", target_bir_lowering=False,
                   num_swdge_queues=N_QUEUES)

    twT = nc.dram_tensor("twT", [128, KT, NP2], DT, kind="ExternalInput")
    idx16 = nc.dram_tensor("idx16", [128, TIDX // 16], mybir.dt.int16,
                           kind="ExternalInput")
    slotw = nc.dram_tensor("slotw", [128, TC, 4], F32, kind="ExternalInput")
    iota = nc.dram_tensor("iota", [128, R * WIN], BF16, kind="ExternalInput")
    ident = nc.dram_tensor("ident", [128, 128], DT, kind="ExternalInput")
    wt = nc.dram_tensor("wt", [128, KT, 128], DT, kind="ExternalInput")
    bt = nc.dram_tensor("bt", [128, 1], F32, kind="ExternalInput")
    win = nc.dram_tensor("win", [128, 128], DT, kind="ExternalInput")
    bin_ = nc.dram_tensor("bin", [128, 1], F32, kind="ExternalInput")
    wr = nc.dram_tensor("wr", [128, R * 128], DT, kind="ExternalInput")
    root = nc.dram_tensor("root", [128, 128], DT, kind="ExternalInput")
    brg = nc.dram_tensor("brg", [128, 1], F32, kind="ExternalInput")
    w1 = nc.dram_tensor("w1", [128, 128], DT, kind="ExternalInput")
    b1 = nc.dram_tensor("b1", [128, 1], F32, kind="ExternalInput")
    w2 = nc.dram_tensor("w2", [128, OUT], DT, kind="ExternalInput")
    b2 = nc.dram_tensor("b2", [OUT, 1], F32, kind="ExternalInput")
    outT = nc.dram_tensor("outT", [OUT, NP2], F32, kind="ExternalOutput")

    with tile.TileContext(nc) as tc:
        nc.gpsimd.load_library(library_config.mlp)
        with ExitStack() as stack:
            cpool = stack.enter_context(tc.tile_pool(name="const", bufs=1))
            dpool = stack.enter_context(
                tc.tile_pool(name="dram", bufs=1, space="DRAM"))
            persist = stack.enter_context(tc.tile_pool(name="persist", bufs=1))

            def cload(dram_t, shape, dtype):
                t = cpool.tile(shape, dtype, name=f"c_{dram_t.name}")
                nc.sync.dma_start(t[:], dram_t[:])
                return t

            wt_s = cload(wt, [128, KT, 128], DT)
            bt_s = cload(bt, [128, 1], F32)
            win_s = cload(win, [128, 128], DT)
            bin_s = cload(bin_, [128, 1], F32)
            wr_s = cload(wr, [128, R * 128], DT)
            root_s = cload(root, [128, 128], DT)
            brg_s = cload(brg, [128, 1], F32)
            w1_s = cload(w1, [128, 128], DT)
            b1_s = cload(b1, [128, 1], F32)
            w2_s = cload(w2, [128, OUT], DT)
            b2_s = cload(b2, [OUT, 1], F32)
            idx_s = cload(idx16, [128, TIDX // 16], mybir.dt.int16)
            slotw_s = cload(slotw, [128, TC, 4], F32)
            iota_s = cload(iota, [128, R * WIN], BF16)
            ident_s = cload(ident, [128, 128], DT)

            tables = [dpool.tile([C * NP2, 128], BF16, addr_space="Shared",
                                 name=f"table{i}") for i in range(2)]
            agin = dpool.tile([NP2, 128], BF16, name="agin")

            xT = persist.tile([128, NP2], DT, name="xT")

            gsems = [nc.alloc_semaphore(f"gsem{q}") for q in range(N_QUEUES)]
            pe_done = nc.alloc_semaphore("pe_done")
            for q in range(N_QUEUES):
                nc.gpsimd.sem_clear(gsems[q])
            nc.gpsimd.sem_clear(pe_done)
            gcount = [0] * N_QUEUES
            sg_done = [0]  # supergroups fully consumed by the PE so far

            # ---------------- stage 1: x = lrelu(lrelu(tweet@Wt+bt)@Win+bin)
            with tc.tile_pool(name="s1", bufs=2) as s1p, \
                 tc.tile_pool(name="s1t", bufs=4) as ttp, \
                 tc.tile_pool(name="ps1", bufs=4, space="PSUM") as ps1:
                for (c0, fw) in _col_tiles(NP2, FW0):
                    twt = s1p.tile([128, KT, fw], DT, tag="twt", name="twt")
                    nc.sync.dma_start(twt[:], twT[:, :, c0:c0 + fw])
                    for (c1, fw2) in _col_tiles(fw, 512):
                        ps_t = ps1.tile([128, fw2], F32, tag="pst", name="ps_t")
                        for k in range(KT):
                            nc.tensor.matmul(ps_t[:], wt_s[:, k, :],
                                             twt[:, k, c1:c1 + fw2],
                                             start=(k == 0), stop=(k == KT - 1))
                        tt = ttp.tile([128, fw2], DT, tag="tt", name="tt")
                        nc.scalar.activation(tt[:], ps_t[:], AF.Lrelu,
                                             bias=bt_s[:], alpha=0.01)
                        ps_x = ps1.tile([128, fw2], F32, tag="psx", name="ps_x")
                        nc.tensor.matmul(ps_x[:], win_s[:], tt[:],
                                         start=True, stop=True)
                        nc.scalar.activation(xT[:, c0 + c1:c0 + c1 + fw2],
                                             ps_x[:], AF.Lrelu,
                                             bias=bin_s[:], alpha=0.01)

            # ---------------- 2 RGCN layers
            for layer in range(2):
                table = tables[layer]
                # phase A: transpose x windows to node-major + AllGather
                with tc.tile_pool(name=f"pa{layer}", bufs=3) as pap, \
                     tc.tile_pool(name=f"psa{layer}", bufs=2,
                                  space="PSUM") as psa:
                    for nt in range(NW):
                        psT = psa.tile([128, 128], DT, tag="psT", name="psT")
                        nc.tensor.transpose(psT[:],
                                            xT[:, nt * 128:(nt + 1) * 128],
                                            ident_s[:])
                        ob = pap.tile([128, 128], BF16, tag="ob", name="ob")
                        nc.scalar.activation(ob[:], psT[:], AF.Copy)
                        nc.sync.dma_start(
                            agin[nt * 128:(nt + 1) * 128, :], ob[:])
                    nc.gpsimd.collective_compute(
                        "AllGather", mybir.AluOpType.bypass,
                        replica_groups=[list(range(C))],
                        ins=[agin[:]],
                        outs=[table[:]])
                    # Pool-queue probe read of the table: its wait on the
                    # collective blocks the Pool sequencer, so the deferred
                    # gather DMAs (triggered below) cannot start early.
                    probe = pap.tile([128, 2], BF16, tag="probe",
                                     name="probe")
                    probe_bi = nc.gpsimd.dma_start(probe[:], table[0:128, 0:2])

                # phase B: gather + on-device one-hot scatter matmuls
                per_sg = {}
                for (b, gc0, nch, s0) in gather_insts:
                    per_sg[s0] = per_sg.get(s0, 0) + 1
                g_bufs = G_BUFS or ((PIPE_DEPTH + 1) * max(per_sg.values()) + 4)
                with tc.tile_pool(name=f"g{layer}", bufs=g_bufs) as gp, \
                     tc.tile_pool(name=f"s{layer}", bufs=S_BUFS) as sp, \
                     tc.tile_pool(name=f"u{layer}", bufs=S_BUFS) as up, \
                     tc.tile_pool(name=f"m{layer}", bufs=M_BUFS) as mp, \
                     tc.tile_pool(name=f"pb{layer}", bufs=PS_BUFS,
                                  space="PSUM") as pb, \
                     tc.tile_pool(name=f"pc{layer}", bufs=PSB_BUFS,
                                  space="PSUM") as pc:
                    by_sg = {}
                    qload = [0] * N_QUEUES
                    for (b, gc0, nch, s0) in gather_insts:
                        qi = qload.index(min(qload))
                        qload[qi] += nch
                        by_sg.setdefault(s0, []).append((b, gc0, nch, qi))
                    sidx = 0
                    prev_triggers = []
                    prev_inc = None
                    for si, s0 in enumerate(range(0, NW, SG_WINDOWS)):
                        vs = range(s0, min(s0 + SG_WINDOWS, NW))
                        gts = {}
                        used_q = set()
                        gate = sg_done[0] + si - PIPE_DEPTH
                        gate_bi = None
                        if PREP_ONLY and gate > 0:
                            # don't run more than PIPE_DEPTH supergroups ahead
                            # of the PE: bounds gather-buffer reuse (WAR)
                            gate_bi = nc.gpsimd.wait_ge(pe_done, gate)
                            _order(gate_bi, prev_triggers)
                        for (b, gc0, nch, qi) in by_sg.get(s0, []):
                            gt = gp.tile([128, nch, 128], BF16, tag="g",
                                         name="gt")
                            if PREP_ONLY:
                                p_bi = nc.gpsimd.dma_gather(
                                    gt[:], table[b * NP2:(b + 1) * NP2, :],
                                    idx_s[:, gc0 * 8: (gc0 + nch) * 8],
                                    nch * WIN, nch * WIN, 128,
                                    single_packet=SINGLE_PACKET,
                                    prepare_only=True, sem=gsems[qi],
                                    queue_num=qi)
                                _order(p_bi, [probe_bi, gate_bi])
                                used_q.add(qi)
                                gcount[qi] += 1
                            else:
                                nc.gpsimd.dma_gather(
                                    gt[:], table[b * NP2:(b + 1) * NP2, :],
                                    idx_s[:, gc0 * 8: (gc0 + nch) * 8],
                                    nch * WIN, nch * WIN, 128,
                                    single_packet=SINGLE_PACKET,
                                    queue_num=qi)
                            gts.setdefault(b, []).append((gt, gc0, nch))
                        triggers = []
                        waits = []
                        for qi in sorted(used_q):
                            triggers.append(nc.gpsimd.trigger_dma(
                                count=None, queue_num=qi))
                        for qi in sorted(used_q):
                            # user-synced gate: PE proceeds only once every
                            # gather of this supergroup has landed in SBUF
                            w_bi = nc.tensor.wait_ge(gsems[qi],
                                                     16 * gcount[qi])
                            _order(w_bi, triggers + [prev_inc])
                            waits.append(w_bi)
                        sg_mms = []
                        for v in vs:
                            nch_v = int(K[v].sum())
                            psB = pc.tile([128, WIN], F32, tag="psb",
                                          name="psB")
                            if nch_v:
                                psA = pb.tile([128, R * WIN], F32, tag="psA",
                                              name="psA")
                                i = 0
                                for b in range(C):
                                    for k in range(int(K[v, b])):
                                        cg = int(go_base[v, b]) + k
                                        ccx = int(co_base[v, b]) + k
                                        gt = None
                                        for (g_t, g_0, g_n) in gts[b]:
                                            if g_0 <= cg < g_0 + g_n:
                                                gt, j = g_t, cg - g_0
                                                break
                                        st = sp.tile([128, R * WIN], BF16,
                                                     tag="S", name="st")
                                        if sidx % SCALAR_MOD != SCALAR_MOD - 1:
                                            nc.vector.tensor_scalar(
                                                st[:], iota_s[:],
                                                slotw_s[:, ccx, 0:1],
                                                slotw_s[:, ccx, 1:2],
                                                op0=ALU.is_equal, op1=ALU.mult)
                                        else:
                                            ut = up.tile([128, R * WIN], BF16,
                                                         tag="u", name="ut")
                                            nc.scalar.activation(
                                                ut[:], iota_s[:], AF.Abs,
                                                bias=slotw_s[:, ccx, 2:3])
                                            nc.scalar.activation(
                                                st[:], ut[:], AF.Relu,
                                                bias=slotw_s[:, ccx, 1:2],
                                                scale=slotw_s[:, ccx, 3:4])
                                        sidx += 1
                                        mm_bi = nc.tensor.matmul(
                                            psA[:], gt[:, j, :], st[:],
                                            start=(i == 0),
                                            stop=(i == nch_v - 1))
                                        if PREP_ONLY:
                                            _order(mm_bi, waits)
                                            sg_mms.append(mm_bi)
                                        i += 1
                                m = mp.tile([128, R * WIN], BF16, tag="m",
                                            name="m")
                                nc.scalar.activation(m[:], psA[:], AF.Copy)
                                nc.tensor.matmul(psB[:], wr_s[:, 0:128],
                                                 m[:, 0:128],
                                                 start=True, stop=False)
                                nc.tensor.matmul(psB[:], wr_s[:, 128:256],
                                                 m[:, 128:256],
                                                 start=False, stop=False,
                                                 skip_group_check=True)
                                nc.tensor.matmul(psB[:], root_s[:],
                                                 xT[:, v * 128:(v + 1) * 128],
                                                 start=False, stop=True,
                                                 skip_group_check=True)
                            else:
                                nc.tensor.matmul(psB[:], root_s[:],
                                                 xT[:, v * 128:(v + 1) * 128],
                                                 start=True, stop=True)
                            nc.vector.tensor_scalar(
                                xT[:, v * 128:(v + 1) * 128], psB[:],
                                brg_s[:], None, op0=ALU.add)
                        if PREP_ONLY:
                            inc_bi = nc.tensor.sem_inc(pe_done, 1)
                            _order(inc_bi, sg_mms + waits)
                            prev_inc = inc_bi
                            prev_triggers = triggers
                    sg_done[0] += len(range(0, NW, SG_WINDOWS))

            # ---------------- head
            with tc.tile_pool(name="hd", bufs=3) as hp, \
                 tc.tile_pool(name="psh", bufs=2, space="PSUM") as psh, \
                 tc.tile_pool(name="outp", bufs=1) as outp:
                outT_s = outp.tile([OUT, NP2], F32, name="outT_s")
                for (c0, fw) in _col_tiles(NP2, 512):
                    ph = psh.tile([128, fw], F32, tag="ph", name="ph")
                    nc.tensor.matmul(ph[:], w1_s[:], xT[:, c0:c0 + fw],
                                     start=True, stop=True)
                    ht = hp.tile([128, fw], DT, tag="ht", name="ht")
                    nc.scalar.activation(ht[:], ph[:], AF.Lrelu,
                                         bias=b1_s[:], alpha=0.01)
                    po = psh.tile([OUT, fw], F32, tag="po", name="po")
                    nc.tensor.matmul(po[:], w2_s[:], ht[:],
                                     start=True, stop=True)
                    nc.vector.tensor_scalar(outT_s[:, c0:c0 + fw], po[:],
                                            b2_s[:], None, op0=ALU.add)
                nc.sync.dma_start(outT[:, :], outT_s[:])

    nc.compile()
    return nc


# ----------------------------------------------------------------------------
# entry point
# ----------------------------------------------------------------------------

def kernel(**inputs):
    global LAST_RESULTS
    tweet = np.asarray(inputs["tweet"], np.float32)
    ei = np.asarray(inputs["edge_index"]).astype(np.int64)
    et = np.asarray(inputs["edge_type"]).astype(np.int64)
    W_tweet = np.asarray(inputs["W_tweet"], np.float32)
    b_tweet = np.asarray(inputs["b_tweet"], np.float32)
    W_in = np.asarray(inputs["W_in"], np.float32)
    b_in = np.asarray(inputs["b_in"], np.float32)
    rgcn_weight = np.asarray(inputs["rgcn_weight"], np.float32)
    rgcn_root = np.asarray(inputs["rgcn_root"], np.float32)
    rgcn_bias = np.asarray(inputs["rgcn_bias"], np.float32)
    W_out1 = np.asarray(inputs["W_out1"], np.float32)
    b_out1 = np.asarray(inputs["b_out1"], np.float32)
    W_out2 = np.asarray(inputs["W_out2"], np.float32)
    b_out2 = np.asarray(inputs["b_out2"], np.float32)

    N, TW = tweet.shape
    D = W_in.shape[0]
    OUT = W_out2.shape[1]
    assert N % C == 0 and TW % 128 == 0 and D == 128
    NPC = N // C
    NP2 = -(-NPC // WIN) * WIN
    src, dst = ei[0], ei[1]

    meta = _edge_meta(src, dst, et, N, NPC, NP2)
    shapes = dict(N=N, TW=TW, D=D, OUT=OUT, NPC=NPC, NP2=NP2)
    npdt = _np_dt()
    KT = TW // 128

    nc = _build_program(shapes, meta)

    shared = {
        "iota": np.tile(np.arange(R * WIN, dtype=np.float32)
                        .astype(ml_dtypes.bfloat16), (128, 1)),
        "ident": np.eye(128, dtype=np.float32).astype(npdt),
        "wt": np.ascontiguousarray(
            W_tweet.reshape(KT, 128, 128).transpose(1, 0, 2)).astype(npdt),
        "bt": b_tweet.reshape(128, 1),
        "win": W_in.astype(npdt),
        "bin": b_in.reshape(128, 1),
        "wr": np.ascontiguousarray(
            rgcn_weight.transpose(1, 0, 2).reshape(128, R * 128)).astype(npdt),
        "root": rgcn_root.astype(npdt),
        "brg": rgcn_bias.reshape(128, 1),
        "w1": W_out1.astype(npdt),
        "b1": b_out1.reshape(128, 1),
        "w2": W_out2.astype(npdt),
        "b2": b_out2.reshape(OUT, 1),
    }

    in_maps = []
    for c in range(C):
        tw_c = np.zeros((128, KT, NP2), npdt)
        tw_c[:, :, :NPC] = (tweet[c * NPC:(c + 1) * NPC].T
                            .reshape(KT, 128, NPC).transpose(1, 0, 2)
                            .astype(npdt))
        m = dict(shared)
        m["twT"] = tw_c
        m["idx16"] = meta["idxw"][c]
        m["slotw"] = meta["slotw"][c]
        in_maps.append(m)

    res = run_bass_kernel_spmd(nc, in_maps, core_ids=list(range(C)),
                               trace=TRACE, tmpdir=TMPDIR)
    LAST_RESULTS = res

    out = np.zeros((N, OUT), np.float32)
    for c in range(C):
        out[c * NPC:(c + 1) * NPC] = res.results[c]["outT"][:, :NPC].T
    return out
